# revision 47
# baseline (speedup 1.0000x reference)
"""Trainium2 Bass/Tile kernel for nn_Attention_50242527428847.

Computation (per batch element b, one NeuronCore each):
    dec[t,e]   = sum_h decoder_states[t,b,h] * W[e,h]            (projection)
    p[t,s,e]   = exp(dec[t,e] * encoder_states[s,b,e])           (softmax numerator over s)
    denom[t,e] = sum_s p[t,s,e]
    wsum[t,s]  = sum_e p[t,s,e] / denom[t,e]
    out[t,b,d] = sum_s wsum[t,s] * encoder_inputs[s,b,d]

Engine assignment (v2 cost model arithmetic, per core):
  - The elementwise triple (multiply, exp, denom-reduce) is 3 x 65536
    elems/partition; exp can only run on ACT (0.833 ns/elem), which sets a
    ~62 us floor.  Everything else is balanced around it (~70 us/engine):
  - ACT: per-(blk,ce)-slab exp over [128, 16, 128] bf16 in-place (~1.9 us
    each), plus 28 "fused" rows exp(enc*scale=dec_col) with accum_out
    producing the denominator directly (absorbs mult+den at ~0.5 us/row).
  - DVE: row-level tensor_scalar multiplies in bf16 (4x perf mode, ~98
    ns/row; accum_out would double an instruction's charge, so denominators
    do NOT ride the rows), denominator fold chains (bf16 halving adds hit
    the 2x_1p mode, then one small fp32 reduce_sum: ~1.44 us/slab, cheaper
    than a flat reduce at 2.24), reciprocals (straight to bf16), copies.
  - GPSIMD: slab-level broadcast multiplies (stride-0 APs) for 17 of 32
    slabs (~4.2 us each); block 0 stays off GPSIMD so its rows start as
    soon as the projection's first columns land.
  - PE: projection, per-(t,ce) N=1 wsum matmuls (lhsT=p chunk, rhs=1/denom
    column), final out = wsum_T.T @ enc_in per block-pair; all matmul
    operands bf16.  Tiny keep-warm matmuls stop the HAM clock-gate from
    re-throttling the PE between bursts.
  - Emission is software-pipelined with a one-block skew (block k+1's
    multiplies before block k's denominators) so no engine's in-order queue
    head-of-line blocks on the previous block's cross-engine chain.

bf16 is used for all bulk tensors (inputs converted host-side); fp32 is kept
for dec (PSUM copies), denominator accumulation, and the final output.
Measured end-to-end rel err ~3e-3 vs the fp32 reference, well under the
2e-2 gate.

Build note: the TRN2 ISA has ONE semaphore wait slot per instruction, so the
program must be built with bacc.Bacc and nc.compile() (its event-semaphore
passes legalize Tile's multi-wait instructions).  Input DMAs are split across
both HW-DGE rings (SP + ACT) and ordered so the projection's inputs land
first.
"""

import numpy as np
from contextlib import ExitStack

import concourse.bass as bass
import concourse.bacc as bacc
import concourse.tile as tile
from concourse import mybir
from concourse.bass_utils import run_bass_kernel_spmd

TD, TE, B = 128, 128, 8
E, H, D = 512, 1024, 256
P = 128
CE = E // P          # 4 e-chunks
CH = H // P          # 8 h-chunks
TB = 16              # t-block size
NBLK = TD // TB      # 8 blocks

# per-slab (blk*CE+ce) tuning knobs:
#   n_fused: rows 0..n_fused-1 of the slab go down the ACT fused+accum path
#   pool_mult: remaining rows' multiply on GPSIMD (slab bcast) vs DVE (rows)
N_FUSED = [0] * (NBLK * CE)
POOL_MULT = [False] * (NBLK * CE)
# block 0 stays off GPSIMD so its rows start as soon as projection columns
# land; POOL_DEN slabs get their denominator via GPSIMD tensor_reduce
# (default-efficiency bucket, 2.85us/slab) instead of DVE fold chains
POOL_DEN = [False] * (NBLK * CE)
_POOL_PER_BLK = [0, 2, 3, 2, 3, 2, 2, 3]
for _b in range(NBLK):
    for _c in range(_POOL_PER_BLK[_b]):
        POOL_MULT[_b * CE + (0, 2, 3, 1)[_c]] = True
for _s in (9, 13, 21, 25):
    N_FUSED[_s] = 7

_F32 = mybir.dt.float32
_BF16 = mybir.dt.bfloat16
_CACHE = {}


def _kernel_body(ctx, tc, out_ap, wt_ap, dtr_ap, et_ap, ei_ap):
    nc = tc.nc
    AF = mybir.ActivationFunctionType

    singles = ctx.enter_context(tc.tile_pool(name="singles", bufs=1))
    p_pool = ctx.enter_context(tc.tile_pool(name="p", bufs=4))
    psum_pool = ctx.enter_context(tc.tile_pool(name="psum", bufs=2, space="PSUM"))
    psum_w = ctx.enter_context(tc.tile_pool(name="psum_w", bufs=2, space="PSUM"))
    psum_o = ctx.enter_context(tc.tile_pool(name="psum_o", bufs=1, space="PSUM"))

    # ---- load inputs (bf16, natural-layout DMAs; transposes host-side)
    # 5 consolidated issues (the HW-DGE issue path serializes at ~630ns
    # apiece), spread over both rings, projection inputs first
    dt_sb = singles.tile([P, CH, TD], _BF16)  # D.T chunks
    dt_r = dtr_ap.rearrange("(c p) t -> p c t", p=P)
    nc.scalar.dma_start(out=dt_sb[:], in_=dt_r[:])
    # wt arrives host-pre-shuffled to the SBUF layout (ce-major so the
    # ce0 slab -- the projection's critical input -- is one contiguous DMA)
    wt_sb = singles.tile([P, CE, CH, P], _BF16)  # [hp, ce, hc, e_local]
    wt_r = wt_ap.rearrange("p (ce c m) -> p ce c m", ce=CE, c=CH)
    nc.sync.dma_start(out=wt_sb[:, 0, :, :], in_=wt_r[:, 0, :, :])
    nc.scalar.dma_start(out=wt_sb[:, 1:, :, :], in_=wt_r[:, 1:, :, :])
    et_sb = singles.tile([P, CE, TE], _BF16)  # enc.T chunks: [e_local, ce, s]
    nc.sync.dma_start(out=et_sb[:], in_=et_ap.rearrange("(c p) s -> p c s", p=P))
    ei_sb = singles.tile([P, D], _BF16)       # enc_in natural [s, d]
    nc.sync.dma_start(out=ei_sb[:], in_=ei_ap)

    # per-block statistics in static tiles (no slot-recycle semaphore waits)
    den_all = singles.tile([P, NBLK, CE, TB], _F32)
    rb_all = singles.tile([P, NBLK, CE, TB], _BF16)
    scratch = singles.tile([P, TB, 64], _BF16)  # DVE den fold workspace
    scratch_p = singles.tile([P, TB, 64], _BF16)  # GPSIMD den fold workspace

    # ---- projection: dec_T[e, t] = sum_h W.T[h, e] * D.T[h, t]
    dec_sb = singles.tile([P, CE, TD], _F32)  # [e_local, ce, t]
    for ce in range(CE):
        dps = psum_pool.tile([P, TD], _F32)
        # ce0's first t-block gates the whole pipeline: compute those 16
        # columns first so downstream engines start earlier
        tranges = (((0, TB), (TB, 2 * TB), (2 * TB, TD)) if ce == 0
                   else ((0, 2 * TB), (2 * TB, TD)))
        for lo, hi in tranges:
            for c in range(CH):
                nc.tensor.matmul(
                    dps[:, lo:hi],
                    lhsT=wt_sb[:, ce, c, :],
                    rhs=dt_sb[:, c, lo:hi],
                    start=(c == 0),
                    stop=(c == CH - 1),
                )
            nc.vector.tensor_copy(dec_sb[:, ce, lo:hi], dps[:, lo:hi])

    # ---- softmax + weighted e-sums, pipelined over t-blocks
    wsum_sb = singles.tile([P, TD], _BF16)   # wsum_T[s, t], filled per block
    # keep-warm pokes: tiny matmuls spread across each block keep the PE HAM
    # clock-gate from re-throttling during the inter-burst waits
    psum_k = ctx.enter_context(tc.tile_pool(name="psum_k", bufs=1, space="PSUM"))
    warm_ps = psum_k.tile([1, NBLK * CE + 8], _F32)
    _warm = [0]

    def pe_warm(col):
        k = _warm[0]
        _warm[0] += 1
        nc.tensor.matmul(warm_ps[0:1, k:k + 1], lhsT=col, rhs=col,
                         start=True, stop=True)

    def emit_mult_exp(blk, p_t):
        """Scores + exp for one block: DVE/Pool multiplies, ACT fused rows,
        ACT slab exps.  Depends only on dec + et + the p tile slot."""
        t0 = blk * TB
        den = den_all[:, blk, :, :]
        for ce in range(CE):
            s_id = blk * CE + ce
            nf = N_FUSED[s_id]

            # fused rows: one ACT instr each does mult+exp+denominator
            for tl in range(nf):
                t = t0 + tl
                nc.scalar.activation(
                    out=p_t[:, ce, tl, :],
                    in_=et_sb[:, ce, :],
                    func=AF.Exp,
                    scale=dec_sb[:, ce, t:t + 1],
                    accum_out=den[:, ce, tl:tl + 1],
                )

            if nf < TB:
                # scores for remaining rows
                if POOL_MULT[s_id]:
                    # slab bcast multiply on GPSIMD via stride-0 APs
                    dslice = dec_sb[:, ce, t0 + nf:t0 + TB]
                    dec_b = bass.AP(
                        tensor=dslice.tensor, offset=dslice.offset,
                        ap=[dslice.ap[0], dslice.ap[1], [0, TE]],
                    )
                    eslice = et_sb[:, ce, :]
                    enc_b = bass.AP(
                        tensor=eslice.tensor, offset=eslice.offset,
                        ap=[eslice.ap[0], [0, TB - nf], eslice.ap[1]],
                    )
                    nc.gpsimd.tensor_mul(p_t[:, ce, nf:TB, :], dec_b, enc_b)
                else:
                    # row-level bf16 tensor_scalar (4x perf mode) on DVE
                    for tl in range(nf, TB):
                        t = t0 + tl
                        nc.vector.tensor_scalar(
                            p_t[:, ce, tl, :], et_sb[:, ce, :],
                            dec_sb[:, ce, t:t + 1], None,
                            op0=mybir.AluOpType.mult,
                        )

                # exp in-place over the non-fused rows
                nc.scalar.activation(
                    out=p_t[:, ce, nf:TB, :], in_=p_t[:, ce, nf:TB, :],
                    func=AF.Exp,
                )
                pe_warm(p_t[:, ce, nf, 0:1])

    def emit_den_wsum(blk, p_t):
        """Per-ce denominator fold, reciprocal, and wsum accumulation.

        Each ce's chain (fold->reduce->recip->wsum matmuls) is emitted
        independently so the wsum accumulation for ce k runs as soon as that
        ce's reciprocal lands, instead of waiting for the whole block; the
        16 PSUM accumulation groups stay open across the ce passes."""
        t0 = blk * TB
        den = den_all[:, blk, :, :]
        wps = psum_w.tile([P, TB], _F32, name=f"wps{blk}", tag="wps")
        for ce in range(CE):
            nf = N_FUSED[blk * CE + ce]
            if POOL_DEN[blk * CE + ce]:
                # halving adds on GPSIMD, final small reduce on DVE
                nr = TB - nf
                foldp = scratch_p[:, 0:nr, :]
                nc.gpsimd.tensor_add(
                    foldp[:, :, 0:64],
                    p_t[:, ce, nf:TB, 0:64], p_t[:, ce, nf:TB, 64:128])
                nc.gpsimd.tensor_add(
                    foldp[:, :, 0:32], foldp[:, :, 0:32], foldp[:, :, 32:64])
                nc.gpsimd.tensor_add(
                    foldp[:, :, 0:16], foldp[:, :, 0:16], foldp[:, :, 16:32])
                nc.vector.reduce_sum(
                    out=den[:, ce, nf:TB], in_=foldp[:, :, 0:16],
                    axis=mybir.AxisListType.X)
            else:
                # denominator via bf16 halving adds (2x_1p: both operands
                # packed) then a small fp32 reduce; rows 0..nf-1 already
                # have den from the ACT fused path
                nr = TB - nf
                fold = scratch[:, 0:nr, :]
                nc.vector.tensor_add(
                    fold[:, :, 0:64],
                    p_t[:, ce, nf:TB, 0:64], p_t[:, ce, nf:TB, 64:128])
                nc.vector.tensor_add(
                    fold[:, :, 0:32], fold[:, :, 0:32], fold[:, :, 32:64])
                nc.vector.tensor_add(
                    fold[:, :, 0:16], fold[:, :, 0:16], fold[:, :, 16:32])
                nc.vector.reduce_sum(
                    out=den[:, ce, nf:TB], in_=fold[:, :, 0:16],
                    axis=mybir.AxisListType.X)
            with nc.allow_low_precision("1/denom bf16 feeds bf16 matmul"):
                nc.vector.reciprocal(out=rb_all[:, blk, ce, :],
                                     in_=den[:, ce, :])
        for tl in range(TB):
            for ce in range(CE):
                nc.tensor.matmul(
                    wps[:, tl:tl + 1],
                    lhsT=p_t[:, ce, tl, :],
                    rhs=rb_all[:, blk, ce, tl:tl + 1],
                    start=(ce == 0),
                    stop=(ce == CE - 1),
                )
        nc.vector.tensor_copy(wsum_sb[:, t0:t0 + TB], wps[:])

    # software-pipelined emission with a one-block skew so each engine's
    # in-order queue never head-of-line blocks on the previous block's
    # cross-engine chain (DVE mults of block k+1 run while ACT exps block k)
    p_tiles = {}
    for blk in range(NBLK):
        p_tiles[blk] = p_pool.tile([P, CE, TB, TE], _BF16, name=f"p_t{blk}",
                                   tag="p_t")
        emit_mult_exp(blk, p_tiles[blk])
        if blk >= 1:
            emit_den_wsum(blk - 1, p_tiles[blk - 1])
    emit_den_wsum(NBLK - 1, p_tiles[NBLK - 1])

    # ---- final: out[t, d] = sum_s wsum_T[s, t] * enc_in[s, d]
    # per pair of t-blocks (M=32, legal PSUM partition offsets) so most of
    # the final matmul/copy/store retires before the last block finishes
    out_ps = psum_o.tile([P, D], _F32)
    out_sb = singles.tile([P, D], _F32)
    for q in range(NBLK // 2):
        t0 = q * 2 * TB
        nc.tensor.matmul(out_ps[t0:t0 + 2 * TB, :],
                         lhsT=wsum_sb[:, t0:t0 + 2 * TB], rhs=ei_sb[:],
                         start=True, stop=True, tile_position=(0, t0))
        nc.vector.tensor_copy(out_sb[t0:t0 + 2 * TB, :], out_ps[t0:t0 + 2 * TB, :])
        nc.sync.dma_start(out=out_ap[t0:t0 + 2 * TB, :], in_=out_sb[t0:t0 + 2 * TB, :])


def build_program():
    if "nc" in _CACHE:
        return _CACHE["nc"]
    nc = bacc.Bacc("TRN2", target_bir_lowering=False, debug=False, num_devices=B)
    wt = nc.dram_tensor("wt", [P, CH * CE * P], _BF16, kind="ExternalInput").ap()
    dtr = nc.dram_tensor("dtr", [H, TD], _BF16, kind="ExternalInput").ap()
    et = nc.dram_tensor("et", [E, TE], _BF16, kind="ExternalInput").ap()
    ei = nc.dram_tensor("ei", [TE, D], _BF16, kind="ExternalInput").ap()
    out = nc.dram_tensor("out", [TD, D], _F32, kind="ExternalOutput").ap()
    with tile.TileContext(nc) as tc:
        with ExitStack() as ctx:
            _kernel_body(ctx, tc, out, wt, dtr, et, ei)
    nc.compile()
    _CACHE["nc"] = nc
    return nc


def make_in_maps(encoder_inputs, encoder_states, decoder_states, W):
    import ml_dtypes
    bf16 = ml_dtypes.bfloat16

    wtt = np.asarray(W, dtype=np.float32).T  # (H, E)
    # [h_local, e_chunk, h_chunk, e_local] flattened to the SBUF layout
    wt_np = np.ascontiguousarray(
        wtt.reshape(CH, P, CE, P).transpose(1, 2, 0, 3)
        .reshape(P, CE * CH * P)).astype(bf16)
    in_maps = []
    for b in range(B):
        in_maps.append({
            "wt": wt_np,
            "dtr": np.ascontiguousarray(decoder_states[:, b, :].T).astype(bf16),
            "et": np.ascontiguousarray(encoder_states[:, b, :].T).astype(bf16),
            "ei": np.ascontiguousarray(encoder_inputs[:, b, :]).astype(bf16),
        })
    return in_maps


def run_on_hw(in_maps, **kwargs):
    nc = build_program()
    return run_bass_kernel_spmd(nc, in_maps, list(range(B)), **kwargs)


def kernel(**inputs):
    encoder_inputs = np.asarray(inputs["encoder_inputs"], dtype=np.float32)
    encoder_states = np.asarray(inputs["encoder_states"], dtype=np.float32)
    decoder_states = np.asarray(inputs["decoder_states"], dtype=np.float32)
    W = np.asarray(inputs["W"], dtype=np.float32)
    in_maps = make_in_maps(encoder_inputs, encoder_states, decoder_states, W)
    res = run_on_hw(in_maps)
    out = np.stack([res.results[b]["out"] for b in range(B)], axis=1)
    return np.ascontiguousarray(out.astype(np.float32))


# revision 50
# speedup vs baseline: 1.0084x; 1.0084x over previous
"""Trainium2 Bass/Tile kernel for nn_Attention_50242527428847.

Computation (per batch element b, one NeuronCore each):
    dec[t,e]   = sum_h decoder_states[t,b,h] * W[e,h]            (projection)
    p[t,s,e]   = exp(dec[t,e] * encoder_states[s,b,e])           (softmax numerator over s)
    denom[t,e] = sum_s p[t,s,e]
    wsum[t,s]  = sum_e p[t,s,e] / denom[t,e]
    out[t,b,d] = sum_s wsum[t,s] * encoder_inputs[s,b,d]

Engine assignment (v2 cost model arithmetic, per core):
  - The elementwise triple (multiply, exp, denom-reduce) is 3 x 65536
    elems/partition; exp can only run on ACT (0.833 ns/elem), which sets a
    ~62 us floor.  Everything else is balanced around it (~70 us/engine):
  - ACT: per-(blk,ce)-slab exp over [128, 16, 128] bf16 in-place (~1.9 us
    each), plus 28 "fused" rows exp(enc*scale=dec_col) with accum_out
    producing the denominator directly (absorbs mult+den at ~0.5 us/row).
  - DVE: row-level tensor_scalar multiplies in bf16 (4x perf mode, ~98
    ns/row; accum_out would double an instruction's charge, so denominators
    do NOT ride the rows), denominator fold chains (bf16 halving adds hit
    the 2x_1p mode, then one small fp32 reduce_sum: ~1.44 us/slab, cheaper
    than a flat reduce at 2.24), reciprocals (straight to bf16), copies.
  - GPSIMD: slab-level broadcast multiplies (stride-0 APs) for 17 of 32
    slabs (~4.2 us each); block 0 stays off GPSIMD so its rows start as
    soon as the projection's first columns land.
  - PE: projection, per-(t,ce) N=1 wsum matmuls (lhsT=p chunk, rhs=1/denom
    column), final out = wsum_T.T @ enc_in per block-pair; all matmul
    operands bf16.  Tiny keep-warm matmuls stop the HAM clock-gate from
    re-throttling the PE between bursts.
  - Emission is software-pipelined with a one-block skew (block k+1's
    multiplies before block k's denominators) so no engine's in-order queue
    head-of-line blocks on the previous block's cross-engine chain.

bf16 is used for all bulk tensors (inputs converted host-side); fp32 is kept
for dec (PSUM copies), denominator accumulation, and the final output.
Measured end-to-end rel err ~3e-3 vs the fp32 reference, well under the
2e-2 gate.

Build note: the TRN2 ISA has ONE semaphore wait slot per instruction, so the
program must be built with bacc.Bacc and nc.compile() (its event-semaphore
passes legalize Tile's multi-wait instructions).  Input DMAs are split across
both HW-DGE rings (SP + ACT) and ordered so the projection's inputs land
first.
"""

import numpy as np
from contextlib import ExitStack

import concourse.bass as bass
import concourse.bacc as bacc
import concourse.tile as tile
from concourse import mybir
from concourse.bass_utils import run_bass_kernel_spmd

TD, TE, B = 128, 128, 8
E, H, D = 512, 1024, 256
P = 128
CE = E // P          # 4 e-chunks
CH = H // P          # 8 h-chunks
TB = 16              # t-block size
NBLK = TD // TB      # 8 blocks

# per-slab (blk*CE+ce) tuning knobs:
#   n_fused: rows 0..n_fused-1 of the slab go down the ACT fused+accum path
#   pool_mult: remaining rows' multiply on GPSIMD (slab bcast) vs DVE (rows)
N_FUSED = [0] * (NBLK * CE)
POOL_MULT = [False] * (NBLK * CE)
# block 0 stays off GPSIMD so its rows start as soon as projection columns
# land; POOL_DEN slabs get their denominator via GPSIMD tensor_reduce
# (default-efficiency bucket, 2.85us/slab) instead of DVE fold chains
POOL_DEN = [False] * (NBLK * CE)
_POOL_PER_BLK = [0, 2, 3, 2, 3, 2, 2, 3]
for _b in range(NBLK):
    for _c in range(_POOL_PER_BLK[_b]):
        POOL_MULT[_b * CE + (0, 2, 3, 1)[_c]] = True
for _s in (9, 13, 21, 25):
    N_FUSED[_s] = 7

_F32 = mybir.dt.float32
_BF16 = mybir.dt.bfloat16
_CACHE = {}


def _kernel_body(ctx, tc, out_ap, wt_ap, dtr_ap, et_ap, ei_ap):
    nc = tc.nc
    AF = mybir.ActivationFunctionType

    singles = ctx.enter_context(tc.tile_pool(name="singles", bufs=1))
    p_pool = ctx.enter_context(tc.tile_pool(name="p", bufs=4))
    psum_pool = ctx.enter_context(tc.tile_pool(name="psum", bufs=2, space="PSUM"))
    psum_w = ctx.enter_context(tc.tile_pool(name="psum_w", bufs=2, space="PSUM"))
    psum_o = ctx.enter_context(tc.tile_pool(name="psum_o", bufs=1, space="PSUM"))

    # ---- load inputs (bf16, natural-layout DMAs; transposes host-side)
    # 5 consolidated issues (the HW-DGE issue path serializes at ~630ns
    # apiece), spread over both rings, projection inputs first
    # dt arrives host-pre-shuffled: per-partition contiguous, one descriptor
    dt_sb = singles.tile([P, CH, TD], _BF16)  # D.T chunks
    nc.scalar.dma_start(out=dt_sb[:], in_=dtr_ap.rearrange("p (f) -> p f"))
    # wt arrives host-pre-shuffled to the SBUF layout (ce-major so the
    # ce0 slab -- the projection's critical input -- is one contiguous DMA)
    wt_sb = singles.tile([P, CE, CH, P], _BF16)  # [hp, ce, hc, e_local]
    wt_r = wt_ap.rearrange("p (ce c m) -> p ce c m", ce=CE, c=CH)
    nc.sync.dma_start(out=wt_sb[:, 0, :, :], in_=wt_r[:, 0, :, :])
    nc.scalar.dma_start(out=wt_sb[:, 1:, :, :], in_=wt_r[:, 1:, :, :])
    et_sb = singles.tile([P, CE, TE], _BF16)  # enc.T chunks: [e_local, ce, s]
    nc.sync.dma_start(out=et_sb[:], in_=et_ap.rearrange("(c p) s -> p c s", p=P))
    ei_sb = singles.tile([P, D], _BF16)       # enc_in natural [s, d]
    nc.sync.dma_start(out=ei_sb[:], in_=ei_ap)

    # per-block statistics in static tiles (no slot-recycle semaphore waits)
    den_all = singles.tile([P, NBLK, CE, TB], _F32)
    rb_all = singles.tile([P, NBLK, CE, TB], _BF16)
    scratch = singles.tile([P, TB, 64], _BF16)  # DVE den fold workspace
    scratch_p = singles.tile([P, TB, 64], _BF16)  # GPSIMD den fold workspace

    # ---- projection: dec_T[e, t] = sum_h W.T[h, e] * D.T[h, t]
    dec_sb = singles.tile([P, CE, TD], _F32)  # [e_local, ce, t]
    for ce in range(CE):
        dps = psum_pool.tile([P, TD], _F32)
        # ce0's first t-block gates the whole pipeline: compute those 16
        # columns first so downstream engines start earlier
        tranges = (((0, TB), (TB, 2 * TB), (2 * TB, TD)) if ce == 0
                   else ((0, 2 * TB), (2 * TB, TD)))
        for lo, hi in tranges:
            for c in range(CH):
                nc.tensor.matmul(
                    dps[:, lo:hi],
                    lhsT=wt_sb[:, ce, c, :],
                    rhs=dt_sb[:, c, lo:hi],
                    start=(c == 0),
                    stop=(c == CH - 1),
                )
            nc.vector.tensor_copy(dec_sb[:, ce, lo:hi], dps[:, lo:hi])

    # ---- softmax + weighted e-sums, pipelined over t-blocks
    wsum_sb = singles.tile([P, TD], _BF16)   # wsum_T[s, t], filled per block
    # keep-warm pokes: tiny matmuls spread across each block keep the PE HAM
    # clock-gate from re-throttling during the inter-burst waits
    psum_k = ctx.enter_context(tc.tile_pool(name="psum_k", bufs=1, space="PSUM"))
    warm_ps = psum_k.tile([1, NBLK * CE + 8], _F32)
    _warm = [0]

    def pe_warm(col):
        k = _warm[0]
        _warm[0] += 1
        nc.tensor.matmul(warm_ps[0:1, k:k + 1], lhsT=col, rhs=col,
                         start=True, stop=True)

    def emit_mult_exp(blk, p_t):
        """Scores + exp for one block: DVE/Pool multiplies, ACT fused rows,
        ACT slab exps.  Depends only on dec + et + the p tile slot."""
        t0 = blk * TB
        den = den_all[:, blk, :, :]
        for ce in range(CE):
            s_id = blk * CE + ce
            nf = N_FUSED[s_id]

            # fused rows: one ACT instr each does mult+exp+denominator
            for tl in range(nf):
                t = t0 + tl
                nc.scalar.activation(
                    out=p_t[:, ce, tl, :],
                    in_=et_sb[:, ce, :],
                    func=AF.Exp,
                    scale=dec_sb[:, ce, t:t + 1],
                    accum_out=den[:, ce, tl:tl + 1],
                )

            if nf < TB:
                # scores for remaining rows
                if POOL_MULT[s_id]:
                    # slab bcast multiply on GPSIMD via stride-0 APs
                    dslice = dec_sb[:, ce, t0 + nf:t0 + TB]
                    dec_b = bass.AP(
                        tensor=dslice.tensor, offset=dslice.offset,
                        ap=[dslice.ap[0], dslice.ap[1], [0, TE]],
                    )
                    eslice = et_sb[:, ce, :]
                    enc_b = bass.AP(
                        tensor=eslice.tensor, offset=eslice.offset,
                        ap=[eslice.ap[0], [0, TB - nf], eslice.ap[1]],
                    )
                    nc.gpsimd.tensor_mul(p_t[:, ce, nf:TB, :], dec_b, enc_b)
                else:
                    # row-level bf16 tensor_scalar (4x perf mode) on DVE
                    for tl in range(nf, TB):
                        t = t0 + tl
                        nc.vector.tensor_scalar(
                            p_t[:, ce, tl, :], et_sb[:, ce, :],
                            dec_sb[:, ce, t:t + 1], None,
                            op0=mybir.AluOpType.mult,
                        )

                # exp in-place over the non-fused rows
                nc.scalar.activation(
                    out=p_t[:, ce, nf:TB, :], in_=p_t[:, ce, nf:TB, :],
                    func=AF.Exp,
                )
                pe_warm(p_t[:, ce, nf, 0:1])

    def emit_den_wsum(blk, p_t):
        """Per-ce denominator fold, reciprocal, and wsum accumulation.

        Each ce's chain (fold->reduce->recip->wsum matmuls) is emitted
        independently so the wsum accumulation for ce k runs as soon as that
        ce's reciprocal lands, instead of waiting for the whole block; the
        16 PSUM accumulation groups stay open across the ce passes."""
        t0 = blk * TB
        den = den_all[:, blk, :, :]
        wps = psum_w.tile([P, TB], _F32, name=f"wps{blk}", tag="wps")
        for ce in range(CE):
            nf = N_FUSED[blk * CE + ce]
            if POOL_DEN[blk * CE + ce]:
                # halving adds on GPSIMD, final small reduce on DVE
                nr = TB - nf
                foldp = scratch_p[:, 0:nr, :]
                nc.gpsimd.tensor_add(
                    foldp[:, :, 0:64],
                    p_t[:, ce, nf:TB, 0:64], p_t[:, ce, nf:TB, 64:128])
                nc.gpsimd.tensor_add(
                    foldp[:, :, 0:32], foldp[:, :, 0:32], foldp[:, :, 32:64])
                nc.gpsimd.tensor_add(
                    foldp[:, :, 0:16], foldp[:, :, 0:16], foldp[:, :, 16:32])
                nc.vector.reduce_sum(
                    out=den[:, ce, nf:TB], in_=foldp[:, :, 0:16],
                    axis=mybir.AxisListType.X)
            else:
                # denominator via bf16 halving adds (2x_1p: both operands
                # packed) then a small fp32 reduce; rows 0..nf-1 already
                # have den from the ACT fused path
                nr = TB - nf
                fold = scratch[:, 0:nr, :]
                nc.vector.tensor_add(
                    fold[:, :, 0:64],
                    p_t[:, ce, nf:TB, 0:64], p_t[:, ce, nf:TB, 64:128])
                nc.vector.tensor_add(
                    fold[:, :, 0:32], fold[:, :, 0:32], fold[:, :, 32:64])
                nc.vector.tensor_add(
                    fold[:, :, 0:16], fold[:, :, 0:16], fold[:, :, 16:32])
                nc.vector.reduce_sum(
                    out=den[:, ce, nf:TB], in_=fold[:, :, 0:16],
                    axis=mybir.AxisListType.X)
            with nc.allow_low_precision("1/denom bf16 feeds bf16 matmul"):
                nc.vector.reciprocal(out=rb_all[:, blk, ce, :],
                                     in_=den[:, ce, :])
        for tl in range(TB):
            for ce in range(CE):
                nc.tensor.matmul(
                    wps[:, tl:tl + 1],
                    lhsT=p_t[:, ce, tl, :],
                    rhs=rb_all[:, blk, ce, tl:tl + 1],
                    start=(ce == 0),
                    stop=(ce == CE - 1),
                )
        nc.vector.tensor_copy(wsum_sb[:, t0:t0 + TB], wps[:])

    # software-pipelined emission with a one-block skew so each engine's
    # in-order queue never head-of-line blocks on the previous block's
    # cross-engine chain (DVE mults of block k+1 run while ACT exps block k)
    p_tiles = {}
    for blk in range(NBLK):
        p_tiles[blk] = p_pool.tile([P, CE, TB, TE], _BF16, name=f"p_t{blk}",
                                   tag="p_t")
        emit_mult_exp(blk, p_tiles[blk])
        if blk >= 1:
            emit_den_wsum(blk - 1, p_tiles[blk - 1])
    emit_den_wsum(NBLK - 1, p_tiles[NBLK - 1])

    # ---- final: out[t, d] = sum_s wsum_T[s, t] * enc_in[s, d]
    # per pair of t-blocks (M=32, legal PSUM partition offsets) so most of
    # the final matmul/copy/store retires before the last block finishes
    out_ps = psum_o.tile([P, D], _F32)
    out_sb = singles.tile([P, D], _F32)
    for q in range(NBLK // 2):
        t0 = q * 2 * TB
        nc.tensor.matmul(out_ps[t0:t0 + 2 * TB, :],
                         lhsT=wsum_sb[:, t0:t0 + 2 * TB], rhs=ei_sb[:],
                         start=True, stop=True, tile_position=(0, t0))
        nc.vector.tensor_copy(out_sb[t0:t0 + 2 * TB, :], out_ps[t0:t0 + 2 * TB, :])
        nc.sync.dma_start(out=out_ap[t0:t0 + 2 * TB, :], in_=out_sb[t0:t0 + 2 * TB, :])


def build_program():
    if "nc" in _CACHE:
        return _CACHE["nc"]
    nc = bacc.Bacc("TRN2", target_bir_lowering=False, debug=False, num_devices=B)
    wt = nc.dram_tensor("wt", [P, CH * CE * P], _BF16, kind="ExternalInput").ap()
    dtr = nc.dram_tensor("dtr", [P, CH * TD], _BF16, kind="ExternalInput").ap()
    et = nc.dram_tensor("et", [E, TE], _BF16, kind="ExternalInput").ap()
    ei = nc.dram_tensor("ei", [TE, D], _BF16, kind="ExternalInput").ap()
    out = nc.dram_tensor("out", [TD, D], _F32, kind="ExternalOutput").ap()
    with tile.TileContext(nc) as tc:
        with ExitStack() as ctx:
            _kernel_body(ctx, tc, out, wt, dtr, et, ei)
    nc.compile()
    _CACHE["nc"] = nc
    return nc


def make_in_maps(encoder_inputs, encoder_states, decoder_states, W):
    import ml_dtypes
    bf16 = ml_dtypes.bfloat16

    wtt = np.asarray(W, dtype=np.float32).T  # (H, E)
    # [h_local, e_chunk, h_chunk, e_local] flattened to the SBUF layout
    wt_np = np.ascontiguousarray(
        wtt.reshape(CH, P, CE, P).transpose(1, 2, 0, 3)
        .reshape(P, CE * CH * P)).astype(bf16)
    in_maps = []
    for b in range(B):
        in_maps.append({
            "wt": wt_np,
            # [h_local, h_chunk, t] flattened to the SBUF layout
            "dtr": np.ascontiguousarray(
                decoder_states[:, b, :].T.reshape(CH, P, TD)
                .transpose(1, 0, 2).reshape(P, CH * TD)).astype(bf16),
            "et": np.ascontiguousarray(encoder_states[:, b, :].T).astype(bf16),
            "ei": np.ascontiguousarray(encoder_inputs[:, b, :]).astype(bf16),
        })
    return in_maps


def run_on_hw(in_maps, **kwargs):
    nc = build_program()
    return run_bass_kernel_spmd(nc, in_maps, list(range(B)), **kwargs)


def kernel(**inputs):
    encoder_inputs = np.asarray(inputs["encoder_inputs"], dtype=np.float32)
    encoder_states = np.asarray(inputs["encoder_states"], dtype=np.float32)
    decoder_states = np.asarray(inputs["decoder_states"], dtype=np.float32)
    W = np.asarray(inputs["W"], dtype=np.float32)
    in_maps = make_in_maps(encoder_inputs, encoder_states, decoder_states, W)
    res = run_on_hw(in_maps)
    out = np.stack([res.results[b]["out"] for b in range(B)], axis=1)
    return np.ascontiguousarray(out.astype(np.float32))


# revision 53
# speedup vs baseline: 1.0107x; 1.0023x over previous
"""Trainium2 Bass/Tile kernel for nn_Attention_50242527428847.

Computation (per batch element b, one NeuronCore each):
    dec[t,e]   = sum_h decoder_states[t,b,h] * W[e,h]            (projection)
    p[t,s,e]   = exp(dec[t,e] * encoder_states[s,b,e])           (softmax numerator over s)
    denom[t,e] = sum_s p[t,s,e]
    wsum[t,s]  = sum_e p[t,s,e] / denom[t,e]
    out[t,b,d] = sum_s wsum[t,s] * encoder_inputs[s,b,d]

Engine assignment (v2 cost model arithmetic, per core):
  - The elementwise triple (multiply, exp, denom-reduce) is 3 x 65536
    elems/partition; exp can only run on ACT (0.833 ns/elem), which sets a
    ~62 us floor.  Everything else is balanced around it (~70 us/engine):
  - ACT: per-(blk,ce)-slab exp over [128, 16, 128] bf16 in-place (~1.9 us
    each), plus 28 "fused" rows exp(enc*scale=dec_col) with accum_out
    producing the denominator directly (absorbs mult+den at ~0.5 us/row).
  - DVE: row-level tensor_scalar multiplies in bf16 (4x perf mode, ~98
    ns/row; accum_out would double an instruction's charge, so denominators
    do NOT ride the rows), denominator fold chains (bf16 halving adds hit
    the 2x_1p mode, then one small fp32 reduce_sum: ~1.44 us/slab, cheaper
    than a flat reduce at 2.24), reciprocals (straight to bf16), copies.
  - GPSIMD: slab-level broadcast multiplies (stride-0 APs) for 17 of 32
    slabs (~4.2 us each); block 0 stays off GPSIMD so its rows start as
    soon as the projection's first columns land.
  - PE: projection, per-(t,ce) N=1 wsum matmuls (lhsT=p chunk, rhs=1/denom
    column), final out = wsum_T.T @ enc_in per block-pair; all matmul
    operands bf16.  Tiny keep-warm matmuls stop the HAM clock-gate from
    re-throttling the PE between bursts.
  - Emission is software-pipelined with a one-block skew (block k+1's
    multiplies before block k's denominators) so no engine's in-order queue
    head-of-line blocks on the previous block's cross-engine chain.

bf16 is used for all bulk tensors (inputs converted host-side); fp32 is kept
for dec (PSUM copies), denominator accumulation, and the final output.
Measured end-to-end rel err ~3e-3 vs the fp32 reference, well under the
2e-2 gate.

Build note: the TRN2 ISA has ONE semaphore wait slot per instruction, so the
program must be built with bacc.Bacc and nc.compile() (its event-semaphore
passes legalize Tile's multi-wait instructions).  Input DMAs are split across
both HW-DGE rings (SP + ACT) and ordered so the projection's inputs land
first.
"""

import numpy as np
from contextlib import ExitStack

import concourse.bass as bass
import concourse.bacc as bacc
import concourse.tile as tile
from concourse import mybir
from concourse.bass_utils import run_bass_kernel_spmd

TD, TE, B = 128, 128, 8
E, H, D = 512, 1024, 256
P = 128
CE = E // P          # 4 e-chunks
CH = H // P          # 8 h-chunks
TB = 16              # t-block size
NBLK = TD // TB      # 8 blocks

# per-slab (blk*CE+ce) tuning knobs:
#   n_fused: rows 0..n_fused-1 of the slab go down the ACT fused+accum path
#   pool_mult: remaining rows' multiply on GPSIMD (slab bcast) vs DVE (rows)
N_FUSED = [0] * (NBLK * CE)
POOL_MULT = [False] * (NBLK * CE)
# block 0 stays off GPSIMD so its rows start as soon as projection columns
# land; POOL_DEN slabs get their denominator via GPSIMD tensor_reduce
# (default-efficiency bucket, 2.85us/slab) instead of DVE fold chains
POOL_DEN = [False] * (NBLK * CE)
_POOL_PER_BLK = [0, 2, 3, 2, 3, 2, 2, 3]
for _b in range(NBLK):
    for _c in range(_POOL_PER_BLK[_b]):
        POOL_MULT[_b * CE + (0, 2, 3, 1)[_c]] = True
for _s in (9, 13, 21, 25):
    N_FUSED[_s] = 7

_F32 = mybir.dt.float32
_BF16 = mybir.dt.bfloat16
_CACHE = {}


def _kernel_body(ctx, tc, out_ap, wt_ap, dtr_ap, et_ap, ei_ap):
    nc = tc.nc
    AF = mybir.ActivationFunctionType

    singles = ctx.enter_context(tc.tile_pool(name="singles", bufs=1))
    p_pool = ctx.enter_context(tc.tile_pool(name="p", bufs=4))
    psum_pool = ctx.enter_context(tc.tile_pool(name="psum", bufs=2, space="PSUM"))
    psum_w = ctx.enter_context(tc.tile_pool(name="psum_w", bufs=2, space="PSUM"))
    psum_o = ctx.enter_context(tc.tile_pool(name="psum_o", bufs=1, space="PSUM"))

    # ---- load inputs (bf16, natural-layout DMAs; transposes host-side)
    # 5 consolidated issues (the HW-DGE issue path serializes at ~630ns
    # apiece), spread over both rings, projection inputs first
    # dt arrives host-pre-shuffled: per-partition contiguous, one descriptor
    dt_sb = singles.tile([P, CH, TD], _BF16)  # D.T chunks
    nc.scalar.dma_start(out=dt_sb[:], in_=dtr_ap.rearrange("p (f) -> p f"))
    # wt arrives host-pre-shuffled to the SBUF layout (ce-major so the
    # ce0 slab -- the projection's critical input -- is one contiguous DMA)
    wt_sb = singles.tile([P, CE, CH, P], _BF16)  # [hp, ce, hc, e_local]
    wt_r = wt_ap.rearrange("p (ce c m) -> p ce c m", ce=CE, c=CH)
    nc.sync.dma_start(out=wt_sb[:, 0, :, :], in_=wt_r[:, 0, :, :])
    nc.scalar.dma_start(out=wt_sb[:, 1:, :, :], in_=wt_r[:, 1:, :, :])
    et_sb = singles.tile([P, CE, TE], _BF16)  # enc.T chunks: [e_local, ce, s]
    nc.sync.dma_start(out=et_sb[:], in_=et_ap.rearrange("p (f) -> p f"))
    ei_sb = singles.tile([P, D], _BF16)       # enc_in natural [s, d]
    nc.sync.dma_start(out=ei_sb[:], in_=ei_ap)

    # per-block statistics in static tiles (no slot-recycle semaphore waits)
    den_all = singles.tile([P, NBLK, CE, TB], _F32)
    rb_all = singles.tile([P, NBLK, CE, TB], _BF16)
    scratch = singles.tile([P, TB, 64], _BF16)  # DVE den fold workspace
    scratch_p = singles.tile([P, TB, 64], _BF16)  # GPSIMD den fold workspace

    # ---- projection: dec_T[e, t] = sum_h W.T[h, e] * D.T[h, t]
    dec_sb = singles.tile([P, CE, TD], _F32)  # [e_local, ce, t]
    for ce in range(CE):
        dps = psum_pool.tile([P, TD], _F32)
        # ce0's first t-block gates the whole pipeline: compute those 16
        # columns first so downstream engines start earlier
        tranges = (((0, TB), (TB, 2 * TB), (2 * TB, TD)) if ce == 0
                   else ((0, 2 * TB), (2 * TB, TD)))
        for lo, hi in tranges:
            for c in range(CH):
                nc.tensor.matmul(
                    dps[:, lo:hi],
                    lhsT=wt_sb[:, ce, c, :],
                    rhs=dt_sb[:, c, lo:hi],
                    start=(c == 0),
                    stop=(c == CH - 1),
                )
            nc.vector.tensor_copy(dec_sb[:, ce, lo:hi], dps[:, lo:hi])

    # ---- softmax + weighted e-sums, pipelined over t-blocks
    wsum_sb = singles.tile([P, TD], _BF16)   # wsum_T[s, t], filled per block
    # keep-warm pokes: tiny matmuls spread across each block keep the PE HAM
    # clock-gate from re-throttling during the inter-burst waits
    psum_k = ctx.enter_context(tc.tile_pool(name="psum_k", bufs=1, space="PSUM"))
    warm_ps = psum_k.tile([1, NBLK * CE + 8], _F32)
    _warm = [0]

    def pe_warm(col):
        k = _warm[0]
        _warm[0] += 1
        nc.tensor.matmul(warm_ps[0:1, k:k + 1], lhsT=col, rhs=col,
                         start=True, stop=True)

    def emit_mult_exp(blk, p_t):
        """Scores + exp for one block: DVE/Pool multiplies, ACT fused rows,
        ACT slab exps.  Depends only on dec + et + the p tile slot."""
        t0 = blk * TB
        den = den_all[:, blk, :, :]
        for ce in range(CE):
            s_id = blk * CE + ce
            nf = N_FUSED[s_id]

            # fused rows: one ACT instr each does mult+exp+denominator
            for tl in range(nf):
                t = t0 + tl
                nc.scalar.activation(
                    out=p_t[:, ce, tl, :],
                    in_=et_sb[:, ce, :],
                    func=AF.Exp,
                    scale=dec_sb[:, ce, t:t + 1],
                    accum_out=den[:, ce, tl:tl + 1],
                )

            if nf < TB:
                # scores for remaining rows
                if POOL_MULT[s_id]:
                    # slab bcast multiply on GPSIMD via stride-0 APs
                    dslice = dec_sb[:, ce, t0 + nf:t0 + TB]
                    dec_b = bass.AP(
                        tensor=dslice.tensor, offset=dslice.offset,
                        ap=[dslice.ap[0], dslice.ap[1], [0, TE]],
                    )
                    eslice = et_sb[:, ce, :]
                    enc_b = bass.AP(
                        tensor=eslice.tensor, offset=eslice.offset,
                        ap=[eslice.ap[0], [0, TB - nf], eslice.ap[1]],
                    )
                    nc.gpsimd.tensor_mul(p_t[:, ce, nf:TB, :], dec_b, enc_b)
                else:
                    # row-level bf16 tensor_scalar (4x perf mode) on DVE
                    for tl in range(nf, TB):
                        t = t0 + tl
                        nc.vector.tensor_scalar(
                            p_t[:, ce, tl, :], et_sb[:, ce, :],
                            dec_sb[:, ce, t:t + 1], None,
                            op0=mybir.AluOpType.mult,
                        )

                # exp in-place over the non-fused rows
                nc.scalar.activation(
                    out=p_t[:, ce, nf:TB, :], in_=p_t[:, ce, nf:TB, :],
                    func=AF.Exp,
                )
                pe_warm(p_t[:, ce, nf, 0:1])

    def emit_den_wsum(blk, p_t):
        """Per-ce denominator fold, reciprocal, and wsum accumulation.

        Each ce's chain (fold->reduce->recip->wsum matmuls) is emitted
        independently so the wsum accumulation for ce k runs as soon as that
        ce's reciprocal lands, instead of waiting for the whole block; the
        16 PSUM accumulation groups stay open across the ce passes."""
        t0 = blk * TB
        den = den_all[:, blk, :, :]
        wps = psum_w.tile([P, TB], _F32, name=f"wps{blk}", tag="wps")
        for ce in range(CE):
            nf = N_FUSED[blk * CE + ce]
            if POOL_DEN[blk * CE + ce]:
                # halving adds on GPSIMD, final small reduce on DVE
                nr = TB - nf
                foldp = scratch_p[:, 0:nr, :]
                nc.gpsimd.tensor_add(
                    foldp[:, :, 0:64],
                    p_t[:, ce, nf:TB, 0:64], p_t[:, ce, nf:TB, 64:128])
                nc.gpsimd.tensor_add(
                    foldp[:, :, 0:32], foldp[:, :, 0:32], foldp[:, :, 32:64])
                nc.gpsimd.tensor_add(
                    foldp[:, :, 0:16], foldp[:, :, 0:16], foldp[:, :, 16:32])
                nc.vector.reduce_sum(
                    out=den[:, ce, nf:TB], in_=foldp[:, :, 0:16],
                    axis=mybir.AxisListType.X)
            else:
                # denominator via bf16 halving adds (2x_1p: both operands
                # packed) then a small fp32 reduce; rows 0..nf-1 already
                # have den from the ACT fused path
                nr = TB - nf
                fold = scratch[:, 0:nr, :]
                nc.vector.tensor_add(
                    fold[:, :, 0:64],
                    p_t[:, ce, nf:TB, 0:64], p_t[:, ce, nf:TB, 64:128])
                nc.vector.tensor_add(
                    fold[:, :, 0:32], fold[:, :, 0:32], fold[:, :, 32:64])
                nc.vector.tensor_add(
                    fold[:, :, 0:16], fold[:, :, 0:16], fold[:, :, 16:32])
                nc.vector.reduce_sum(
                    out=den[:, ce, nf:TB], in_=fold[:, :, 0:16],
                    axis=mybir.AxisListType.X)
            with nc.allow_low_precision("1/denom bf16 feeds bf16 matmul"):
                nc.vector.reciprocal(out=rb_all[:, blk, ce, :],
                                     in_=den[:, ce, :])
        for tl in range(TB):
            for ce in range(CE):
                nc.tensor.matmul(
                    wps[:, tl:tl + 1],
                    lhsT=p_t[:, ce, tl, :],
                    rhs=rb_all[:, blk, ce, tl:tl + 1],
                    start=(ce == 0),
                    stop=(ce == CE - 1),
                )
        nc.vector.tensor_copy(wsum_sb[:, t0:t0 + TB], wps[:])

    # software-pipelined emission with a one-block skew so each engine's
    # in-order queue never head-of-line blocks on the previous block's
    # cross-engine chain (DVE mults of block k+1 run while ACT exps block k)
    p_tiles = {}
    for blk in range(NBLK):
        p_tiles[blk] = p_pool.tile([P, CE, TB, TE], _BF16, name=f"p_t{blk}",
                                   tag="p_t")
        emit_mult_exp(blk, p_tiles[blk])
        if blk >= 1:
            emit_den_wsum(blk - 1, p_tiles[blk - 1])
    emit_den_wsum(NBLK - 1, p_tiles[NBLK - 1])

    # ---- final: out[t, d] = sum_s wsum_T[s, t] * enc_in[s, d]
    # per pair of t-blocks (M=32, legal PSUM partition offsets) so most of
    # the final matmul/copy/store retires before the last block finishes
    out_ps = psum_o.tile([P, D], _F32)
    out_sb = singles.tile([P, D], _F32)
    for q in range(NBLK // 2):
        t0 = q * 2 * TB
        nc.tensor.matmul(out_ps[t0:t0 + 2 * TB, :],
                         lhsT=wsum_sb[:, t0:t0 + 2 * TB], rhs=ei_sb[:],
                         start=True, stop=True, tile_position=(0, t0))
        nc.vector.tensor_copy(out_sb[t0:t0 + 2 * TB, :], out_ps[t0:t0 + 2 * TB, :])
        nc.sync.dma_start(out=out_ap[t0:t0 + 2 * TB, :], in_=out_sb[t0:t0 + 2 * TB, :])


def build_program():
    if "nc" in _CACHE:
        return _CACHE["nc"]
    nc = bacc.Bacc("TRN2", target_bir_lowering=False, debug=False, num_devices=B)
    wt = nc.dram_tensor("wt", [P, CH * CE * P], _BF16, kind="ExternalInput").ap()
    dtr = nc.dram_tensor("dtr", [P, CH * TD], _BF16, kind="ExternalInput").ap()
    et = nc.dram_tensor("et", [P, CE * TE], _BF16, kind="ExternalInput").ap()
    ei = nc.dram_tensor("ei", [TE, D], _BF16, kind="ExternalInput").ap()
    out = nc.dram_tensor("out", [TD, D], _F32, kind="ExternalOutput").ap()
    with tile.TileContext(nc) as tc:
        with ExitStack() as ctx:
            _kernel_body(ctx, tc, out, wt, dtr, et, ei)
    nc.compile()
    _CACHE["nc"] = nc
    return nc


def make_in_maps(encoder_inputs, encoder_states, decoder_states, W):
    import ml_dtypes
    bf16 = ml_dtypes.bfloat16

    wtt = np.asarray(W, dtype=np.float32).T  # (H, E)
    # [h_local, e_chunk, h_chunk, e_local] flattened to the SBUF layout
    wt_np = np.ascontiguousarray(
        wtt.reshape(CH, P, CE, P).transpose(1, 2, 0, 3)
        .reshape(P, CE * CH * P)).astype(bf16)
    in_maps = []
    for b in range(B):
        in_maps.append({
            "wt": wt_np,
            # [h_local, h_chunk, t] flattened to the SBUF layout
            "dtr": np.ascontiguousarray(
                decoder_states[:, b, :].T.reshape(CH, P, TD)
                .transpose(1, 0, 2).reshape(P, CH * TD)).astype(bf16),
            # [e_local, e_chunk, s] flattened to the SBUF layout
            "et": np.ascontiguousarray(
                encoder_states[:, b, :].T.reshape(CE, P, TE)
                .transpose(1, 0, 2).reshape(P, CE * TE)).astype(bf16),
            "ei": np.ascontiguousarray(encoder_inputs[:, b, :]).astype(bf16),
        })
    return in_maps


def run_on_hw(in_maps, **kwargs):
    nc = build_program()
    return run_bass_kernel_spmd(nc, in_maps, list(range(B)), **kwargs)


def kernel(**inputs):
    encoder_inputs = np.asarray(inputs["encoder_inputs"], dtype=np.float32)
    encoder_states = np.asarray(inputs["encoder_states"], dtype=np.float32)
    decoder_states = np.asarray(inputs["decoder_states"], dtype=np.float32)
    W = np.asarray(inputs["W"], dtype=np.float32)
    in_maps = make_in_maps(encoder_inputs, encoder_states, decoder_states, W)
    res = run_on_hw(in_maps)
    out = np.stack([res.results[b]["out"] for b in range(B)], axis=1)
    return np.ascontiguousarray(out.astype(np.float32))
